# revision 1
# baseline (speedup 1.0000x reference)
"""Trainium2 Bass kernel for nn_NodeModel (GNN message passing + 3-layer node MLP).

Strategy (node-parallel, 8 cores):
  - Host: sort edges by destination node, bucket them into 128-node tiles,
    pad each tile's edge list to K_CH chunks of 128 edges. Nodes are sharded
    contiguously across the 8 cores (12544 padded nodes each).
  - Device (per core, per 128-node tile):
      aggT[h, n] = sum_k edge_chunk_k[e, h].T @ onehot(col_local_k)[e, n]
      (one-hot built on DVE via iota==col compare; matmul accumulates in PSUM)
      then fused 3-layer MLP with LayerNorm + shifted-softplus, activations
      kept transposed [h, node]; LN stats computed after a PE transpose to
      [node, h]; scale/shift+softplus fused into one ACT op in [h, node].
  - -log(2) of ssp folded into the next layer's bias (host-precomputed);
    final layer subtracts it explicitly.
"""

import os
import sys

import numpy as np

sys.path.insert(0, "/opt/trn_rl_repo")

import bass_rust as _bass_rust
import ml_dtypes

from concourse import bacc, bass, hw_specs, mybir
from concourse import tile as tile_mod
from concourse.bass_utils import run_bass_kernel_spmd
from concourse.masks import make_identity


class _Bacc(bacc.Bacc):
    """Bacc with the ACT table chooser pinned to the single function set
    that holds Ln+Exp+Copy+Identity. The default greedy chooser alternates
    between per-func sets, costing a ~1.3us ACT_TABLE_LOAD per switch."""

    def insert_act_table_loads(self):
        has_activation = any(
            isinstance(i, mybir.InstActivation)
            for b in self.main_func.blocks
            for i in b.instructions
        )
        if not has_activation:
            return
        keep = "natural_log_exp_and_others"
        tables = [
            (n, (s if n == keep else set()))
            for n, s in hw_specs.get_activation_tables(self.m.arch).items()
        ]
        _bass_rust.insert_act_table_loads(self, tables)


LOG2 = float(np.log(2.0))
N, E, H = 100000, 600000, 128
NC = 8
P = 128
TPC = 98                 # 128-node tiles per core
NPC = TPC * P            # nodes per core (12544)
NPAD = NPC * NC          # padded node count (100352)
NT = NPAD // P           # total node tiles (784)

F32 = mybir.dt.float32
F32R = mybir.dt.float32r
BF16 = mybir.dt.bfloat16

LAST_RESULT = None  # BassKernelResults of the most recent run (for profiling)


def _host_prep(x, edge_index, edge_attr):
    col = np.asarray(edge_index)[1].astype(np.int64)
    ea = np.ascontiguousarray(np.asarray(edge_attr, dtype=np.float32))
    order = np.argsort(col, kind="stable")
    col_s = col[order]
    tile_of = col_s >> 7
    counts = np.bincount(tile_of, minlength=NT)
    K = int(np.ceil(counts.max() / P))
    S = K * P
    starts = np.zeros(NT + 1, np.int64)
    starts[1:] = np.cumsum(counts)
    pos = np.arange(E) - starts[tile_of]
    slot = tile_of * S + pos
    slot_edge = np.zeros(NT * S, np.int64)
    slot_edge[slot] = order
    col_local = np.full(NT * S, 128.0, np.float32)
    col_local[slot] = (col_s & 127).astype(np.float32)
    payload = ea[slot_edge]  # [NT*S, H]

    x_pad = np.zeros((NPAD, H), np.float32)
    x_pad[:N] = np.asarray(x, dtype=np.float32)

    per_core = []
    for c in range(NC):
        r0, r1 = c * TPC * S, (c + 1) * TPC * S
        pay_c = np.ascontiguousarray(
            payload[r0:r1]
            .reshape(TPC, K, P, H)
            .transpose(0, 2, 1, 3)
            .reshape(TPC * P, K * H)
            .astype(ml_dtypes.bfloat16)
        )
        col_c = np.ascontiguousarray(
            col_local[r0:r1].reshape(TPC, K, P).transpose(2, 0, 1).reshape(P, TPC * K)
        )
        xt_c = np.ascontiguousarray(
            x_pad[c * NPC : (c + 1) * NPC]
            .reshape(TPC, P, H)
            .transpose(0, 2, 1)
            .reshape(TPC * P, P)
            .astype(ml_dtypes.bfloat16)
        )
        per_core.append((pay_c, col_c, xt_c))
    return K, per_core


def _build_program(K):
    # Bacc (not raw Bass): its compile pass splits multi-semaphore waits into
    # event-semaphore chains — walrus codegen allows only 1 wait per
    # instruction on this toolchain.
    nc = _Bacc("TRN2", target_bir_lowering=False, debug=False, num_devices=NC)

    edges_h = nc.dram_tensor("edges", [TPC * P, K * P], BF16, kind="ExternalInput")
    cols_h = nc.dram_tensor("cols", [P, TPC * K], F32, kind="ExternalInput")
    xt_h = nc.dram_tensor("xt", [TPC * P, P], BF16, kind="ExternalInput")
    w_h = {
        name: nc.dram_tensor(name, [P, P], BF16, kind="ExternalInput")
        for name in ("w1a", "w1b", "w2", "w3")
    }
    # b1,b2,b3,g1,g2,g3,be1,be2,be3 packed as columns of one tensor (one DMA,
    # one semaphore for every per-partition vector operand).
    vecs_h = nc.dram_tensor("vecs", [P, 9], F32, kind="ExternalInput")
    iota_h = nc.dram_tensor("iota", [P, P], F32, kind="ExternalInput")
    out_h = nc.dram_tensor("out", [TPC * P, P], F32, kind="ExternalOutput")
    VIDX = {n: i for i, n in enumerate(("b1", "b2", "b3", "g1", "g2", "g3", "be1", "be2", "be3"))}

    with tile_mod.TileContext(nc) as tc:
        with (
            tc.tile_pool(name="const", bufs=1) as cpool,
            tc.tile_pool(name="edges", bufs=3) as epool,
            tc.tile_pool(name="xin", bufs=3) as xpool,
            tc.tile_pool(name="sel", bufs=4) as selpool,
            tc.tile_pool(name="work", bufs=3) as wpool,
            tc.tile_pool(name="stats", bufs=6) as spool,
            tc.tile_pool(name="psum", bufs=8, space="PSUM") as ppool,
        ):
            ident = cpool.tile([P, P], F32)
            make_identity(nc, ident[:])

            def transpose(dst_psum, src_sbuf):
                nc.tensor.transpose(dst_psum[:], src_sbuf[:], ident[:])
            iota = cpool.tile_from(iota_h[:])
            cols = cpool.tile_from(cols_h[:])
            W = {k: cpool.tile_from(h[:], name=f"w_{k}") for k, h in w_h.items()}
            vecs = cpool.tile_from(vecs_h[:])
            V = {n: vecs[:, i : i + 1] for n, i in VIDX.items()}
            eps = cpool.tile([P, 1], F32)
            nc.gpsimd.memset(eps[:], 1e-5)
            half = cpool.tile([P, 1], F32)
            nc.gpsimd.memset(half[:], 0.5)

            def layer(zT_psum, b, g, be, out_dtype=BF16):
                """zT_psum: [h_out, n] pre-activation in PSUM.
                Returns ssp(LN(zT + b) * g + be) as [h_out, n] in SBUF,
                including the -log2 shift (ln(0.5*exp(y) + 0.5))."""
                # NOTE: TensorScalar's ISA struct fits only ONE sync wait, so
                # everything here uses tensor_tensor with broadcast [P,1] APs.
                zbT = wpool.tile([P, P], F32, tag="zbT")
                nc.vector.tensor_tensor(
                    zbT[:], zT_psum[:], V[b].to_broadcast([P, P]),
                    op=mybir.AluOpType.add,
                )
                z_rm = ppool.tile([P, P], F32, tag="ps")
                transpose(z_rm, zbT)
                st6 = spool.tile([P, 6], F32, tag="st6")
                nc.vector.bn_stats(st6[:], z_rm[:])
                st2 = spool.tile([P, 2], F32, tag="st2")
                nc.vector.bn_aggr(st2[:], st6[:])
                # rsqrt(var + eps) = exp(-0.5 * ln(var + eps)); no ACT func
                # set holds both Sqrt and a softplus path, but Ln+Exp coexist.
                lnv = spool.tile([P, 1], F32, tag="lnv")
                nc.scalar.activation(
                    lnv[:], st2[:, 1:2], mybir.ActivationFunctionType.Ln,
                    bias=eps[:, 0:1],
                )
                rsig = spool.tile([P, 1], F32, tag="rsig")
                nc.scalar.activation(
                    rsig[:], lnv[:], mybir.ActivationFunctionType.Exp, scale=-0.5
                )
                zc = wpool.tile([P, P], F32, tag="zc")
                nc.vector.tensor_tensor(
                    zc[:], z_rm[:], st2[:, 0:1].to_broadcast([P, P]),
                    op=mybir.AluOpType.subtract,
                )
                zn = wpool.tile([P, P], F32, tag="zn")
                zn_eng = nc.gpsimd if os.environ.get("KERNEL_ZN_GPS", "1") == "1" else nc.vector
                zn_eng.tensor_tensor(
                    zn[:], zc[:], rsig[:, 0:1].to_broadcast([P, P]),
                    op=mybir.AluOpType.mult,
                )
                znT = ppool.tile([P, P], F32, tag="ps")
                transpose(znT, zn)
                # ssp(y) = softplus(y) - log2 = ln(0.5*exp(y) + 0.5), with
                # y = g*zn + be. LN output is bounded (|zn| <= sqrt(127)) so
                # exp cannot overflow.
                ez = wpool.tile([P, P], F32, tag="ez")
                nc.scalar.activation(
                    ez[:],
                    znT[:],
                    mybir.ActivationFunctionType.Exp,
                    bias=V[be],
                    scale=V[g],
                )
                spT = wpool.tile([P, P], out_dtype, tag="spT")
                nc.scalar.activation(
                    spT[:], ez[:], mybir.ActivationFunctionType.Ln,
                    bias=half[:, 0:1], scale=0.5,
                )
                return spT

            sel_eng = nc.gpsimd if os.environ.get("KERNEL_SEL_GPS", "0") == "1" else nc.vector
            n_tiles = int(os.environ.get("KERNEL_TPC", str(TPC)))
            for t in range(n_tiles):
                ed = epool.tile([P, K * P], BF16, tag="ed")
                nc.sync.dma_start(out=ed[:], in_=edges_h[t * P : (t + 1) * P, :])
                xt = xpool.tile([P, P], BF16, tag="xt")
                nc.sync.dma_start(out=xt[:], in_=xt_h[t * P : (t + 1) * P, :])

                aggT = ppool.tile([P, P], F32, tag="ps")
                for k in range(K):
                    sel = selpool.tile([P, P], BF16, tag="sel")
                    sel_eng.tensor_tensor(
                        sel[:],
                        cols[:, t * K + k : t * K + k + 1].to_broadcast([P, P]),
                        iota[:],
                        op=mybir.AluOpType.is_equal,
                    )
                    nc.tensor.matmul(
                        out=aggT[:],
                        lhsT=ed[:, k * P : (k + 1) * P],
                        rhs=sel[:],
                        start=(k == 0),
                        stop=(k == K - 1),
                    )
                aggS = wpool.tile([P, P], BF16, tag="aggS")
                nc.vector.tensor_copy(aggS[:], aggT[:])

                z1T = ppool.tile([P, P], F32, tag="ps")
                nc.tensor.matmul(out=z1T[:], lhsT=W["w1a"][:], rhs=xt[:], start=True, stop=False)
                nc.tensor.matmul(out=z1T[:], lhsT=W["w1b"][:], rhs=aggS[:], start=False, stop=True)
                h1T = layer(z1T, "b1", "g1", "be1")

                z2T = ppool.tile([P, P], F32, tag="ps")
                nc.tensor.matmul(out=z2T[:], lhsT=W["w2"][:], rhs=h1T[:], start=True, stop=True)
                h2T = layer(z2T, "b2", "g2", "be2")

                z3T = ppool.tile([P, P], F32, tag="ps")
                nc.tensor.matmul(out=z3T[:], lhsT=W["w3"][:], rhs=h2T[:], start=True, stop=True)
                h3T = layer(z3T, "b3", "g3", "be3", out_dtype=F32)
                nc.sync.dma_start(out=out_h[t * P : (t + 1) * P, :], in_=h3T[:])

    if not nc.is_finalized():
        nc.finalize()
    return nc


def kernel(
    x, edge_index, edge_attr,
    W1, b1, g1, be1, W2, b2, g2, be2, W3, b3, g3, be3,
):
    global LAST_RESULT
    W1 = np.asarray(W1, np.float32)
    W2 = np.asarray(W2, np.float32)
    W3 = np.asarray(W3, np.float32)

    K, per_core = _host_prep(x, edge_index, edge_attr)
    nc = _build_program(K)

    vecs = np.stack(
        [np.asarray(v, np.float32) for v in (b1, b2, b3, g1, g2, g3, be1, be2, be3)],
        axis=1,
    )  # [128, 9], column order must match VIDX in _build_program
    shared = {
        "w1a": np.ascontiguousarray(W1[:P]).astype(ml_dtypes.bfloat16),
        "w1b": np.ascontiguousarray(W1[P:]).astype(ml_dtypes.bfloat16),
        "w2": W2.astype(ml_dtypes.bfloat16),
        "w3": W3.astype(ml_dtypes.bfloat16),
        "vecs": np.ascontiguousarray(vecs),
        "iota": np.ascontiguousarray(
            np.broadcast_to(np.arange(P, dtype=np.float32), (P, P))
        ),
    }
    in_maps = [
        {"edges": pay_c, "cols": col_c, "xt": xt_c, **shared}
        for (pay_c, col_c, xt_c) in per_core
    ]

    trace = bool(int(os.environ.get("KERNEL_TRACE", "0")))
    res = run_bass_kernel_spmd(nc, in_maps, core_ids=list(range(NC)), trace=trace)
    LAST_RESULT = res

    out = np.concatenate(
        [
            r["out"].reshape(TPC, P, P).transpose(0, 2, 1).reshape(NPC, H)
            for r in res.results
        ],
        axis=0,
    )
    return np.ascontiguousarray(out[:N])



# revision 4
# speedup vs baseline: 1.4772x; 1.4772x over previous
"""Trainium2 Bass kernel for nn_NodeModel (GNN message passing + 3-layer node MLP).

Strategy (node-parallel, 8 cores), v2 — transpose-free [h, node] dataflow:
  - Host: sort edges by destination node, bucket into 128-node tiles, pad each
    tile's edge list to K chunks of 128 edges. Nodes sharded contiguously
    across 8 cores (12800 padded nodes each, 25 super-tiles of 512).
  - LayerNorm mean is folded into the weights on host (W' = W - rowmean(W),
    b' = b - mean(b)) so every matmul output is already mean-centered.
  - Per 512-node super-tile on device:
      * aggregation: one-hot sel built by tensor_scalar(iota == col) on
        DVE/GPSIMD, chunk matmuls accumulate aggT[h, n] in PSUM.
      * per layer: bias seeded into PSUM by a rank-1 matmul (b'_row x ones),
        z_c = W'.T @ aT accumulated on top; var broadcast to all partitions
        via an all-ones stationary matmul of sq = z_c^2; rsig = exp(-.5*ln(.));
        zn = z_c * rsig; ssp via two ACT ops Exp(g*x+be), Ln(.5x+.5).
    Everything stays [h, node]; no PE transposes, no bn_stats.
  - ssp's -log2 appears exactly as ln(0.5 e^y + 0.5); output written bf16.
"""

import os
import sys

import numpy as np

sys.path.insert(0, "/opt/trn_rl_repo")

import bass_rust as _bass_rust
import ml_dtypes

from concourse import bacc, bass, hw_specs, mybir
from concourse import tile as tile_mod
from concourse.bass_utils import run_bass_kernel_spmd


class _Bacc(bacc.Bacc):
    """Bacc with the ACT table chooser pinned to the single function set
    that holds Ln+Exp+Copy+Identity. The default greedy chooser alternates
    between per-func sets, costing a ~1.3us ACT_TABLE_LOAD per switch."""

    def insert_act_table_loads(self):
        has_activation = any(
            isinstance(i, mybir.InstActivation)
            for b in self.main_func.blocks
            for i in b.instructions
        )
        if not has_activation:
            return
        keep = "natural_log_exp_and_others"
        tables = [
            (n, (s if n == keep else set()))
            for n, s in hw_specs.get_activation_tables(self.m.arch).items()
        ]
        _bass_rust.insert_act_table_loads(self, tables)


LOG2 = float(np.log(2.0))
N, E, H = 100000, 600000, 128
NC = 8
P = 128
SN = 512                 # nodes per super-tile
TPS = SN // P            # 128-node tiles per super-tile (4)
SPC = 25                 # super-tiles per core
TPC = SPC * TPS          # 128-node tiles per core (100)
NPC = TPC * P            # nodes per core (12800)
NPAD = NPC * NC          # padded node count (102400)
NT = NPAD // P           # total node tiles (800)

F32 = mybir.dt.float32
BF16 = mybir.dt.bfloat16

LAST_RESULT = None  # BassKernelResults of the most recent run (for profiling)


def _host_prep(x, edge_index, edge_attr):
    col = np.asarray(edge_index)[1].astype(np.int64)
    ea = np.ascontiguousarray(np.asarray(edge_attr, dtype=np.float32))
    order = np.argsort(col, kind="stable")
    col_s = col[order]
    tile_of = col_s >> 7
    counts = np.bincount(tile_of, minlength=NT)
    K = int(np.ceil(counts.max() / P))
    S = K * P
    starts = np.zeros(NT + 1, np.int64)
    starts[1:] = np.cumsum(counts)
    pos = np.arange(E) - starts[tile_of]
    slot = tile_of * S + pos
    slot_edge = np.zeros(NT * S, np.int64)
    slot_edge[slot] = order
    col_local = np.full(NT * S, 128.0, np.float32)
    col_local[slot] = (col_s & 127).astype(np.float32)
    payload = ea[slot_edge]  # [NT*S, H]

    x_pad = np.zeros((NPAD, H), np.float32)
    x_pad[:N] = np.asarray(x, dtype=np.float32)

    per_core = []
    for c in range(NC):
        r0, r1 = c * TPC * S, (c + 1) * TPC * S
        # ed rows = edge position within chunk, cols = (tile-chunk, h)
        pay_c = np.ascontiguousarray(
            payload[r0:r1]
            .reshape(TPC, K, P, H)
            .transpose(0, 2, 1, 3)
            .reshape(TPC * P, K * H)
            .astype(ml_dtypes.bfloat16)
        )
        col_c = np.ascontiguousarray(
            col_local[r0:r1].reshape(TPC, K, P).transpose(2, 0, 1).reshape(P, TPC * K)
        )
        # xT: [h, node] per core
        xt_c = np.ascontiguousarray(
            x_pad[c * NPC : (c + 1) * NPC].T.astype(ml_dtypes.bfloat16)
        )
        per_core.append((pay_c, col_c, xt_c))
    return K, per_core


def _build_program(K):
    # Bacc (not raw Bass): its compile pass splits multi-semaphore waits into
    # event-semaphore chains — walrus codegen allows only 1 wait per
    # instruction on this toolchain.
    nc = _Bacc("TRN2", target_bir_lowering=False, debug=False, num_devices=NC)

    edges_h = nc.dram_tensor("edges", [TPC * P, K * P], BF16, kind="ExternalInput")
    cols_h = nc.dram_tensor("cols", [P, TPC * K], F32, kind="ExternalInput")
    xt_h = nc.dram_tensor("xt", [P, NPC], BF16, kind="ExternalInput")
    w_h = {
        name: nc.dram_tensor(name, [P, P], BF16, kind="ExternalInput")
        for name in ("w1a", "w1b", "w2", "w3")
    }
    # bias rows b1',b2',b3' (already mean-centered) as [1,128] for rank-1 seed
    brow_h = {
        i: nc.dram_tensor(f"b{i}", [1, P], BF16, kind="ExternalInput")
        for i in (1, 2, 3)
    }
    # g1..g3, be1..be3 packed as columns of one tensor
    vecs_h = nc.dram_tensor("vecs", [P, 6], F32, kind="ExternalInput")
    iota_h = nc.dram_tensor("iota", [P, P], BF16, kind="ExternalInput")
    out_h = nc.dram_tensor("out", [P, NPC], BF16, kind="ExternalOutput")
    VIDX = {n: i for i, n in enumerate(("g1", "g2", "g3", "be1", "be2", "be3"))}

    sel_dve = int(os.environ.get("KERNEL_SEL_DVE", "4"))  # of every 7 chunks
    n_st = int(os.environ.get("KERNEL_SPC", str(SPC)))

    with tile_mod.TileContext(nc) as tc:
        with (
            tc.tile_pool(name="const", bufs=1) as cpool,
            tc.tile_pool(name="edges", bufs=8) as epool,
            tc.tile_pool(name="xin", bufs=3) as xpool,
            tc.tile_pool(name="sel", bufs=12) as selpool,
            tc.tile_pool(name="aggs", bufs=3) as apool,
            tc.tile_pool(name="sq", bufs=3) as sqpool,
            tc.tile_pool(name="rs", bufs=3) as rspool,
            tc.tile_pool(name="zn", bufs=3) as znpool,
            tc.tile_pool(name="ez", bufs=3) as ezpool,
            tc.tile_pool(name="hout", bufs=4) as hpool,
            tc.tile_pool(name="psagg", bufs=2, space="PSUM") as pagg,
            tc.tile_pool(name="psz", bufs=3, space="PSUM") as psz,
            tc.tile_pool(name="psvar", bufs=3, space="PSUM") as psvar,
        ):
            iota = cpool.tile_from(iota_h[:])
            cols = cpool.tile_from(cols_h[:])
            W = {k: cpool.tile_from(h[:], name=f"w_{k}") for k, h in w_h.items()}
            brow = {i: cpool.tile_from(h[:], name=f"b_{i}") for i, h in brow_h.items()}
            vecs = cpool.tile_from(vecs_h[:])
            V = {n: vecs[:, i : i + 1] for n, i in VIDX.items()}
            ones_rep = cpool.tile([P, P], BF16)
            nc.gpsimd.memset(ones_rep[:], 1.0)
            ones_row = cpool.tile([1, SN], BF16)
            nc.gpsimd.memset(ones_row[:], 1.0)
            eps = cpool.tile([P, 1], F32)
            nc.gpsimd.memset(eps[:], 1e-5)
            half = cpool.tile([P, 1], F32)
            nc.gpsimd.memset(half[:], 0.5)

            for st in range(n_st):
                xT = xpool.tile([P, SN], BF16, tag="xT")
                nc.sync.dma_start(out=xT[:], in_=xt_h[:, st * SN : (st + 1) * SN])

                agg = pagg.tile([P, SN], F32, tag="agg")
                for t4 in range(TPS):
                    t = st * TPS + t4
                    ed = epool.tile([P, K * P], BF16, tag="ed")
                    nc.sync.dma_start(out=ed[:], in_=edges_h[t * P : (t + 1) * P, :])
                    for k in range(K):
                        c = t * K + k
                        sel = selpool.tile([P, P], BF16, tag="sel")
                        eng = nc.vector if k < sel_dve else nc.gpsimd
                        eng.tensor_scalar(
                            sel[:], iota[:], cols[:, c : c + 1], None,
                            op0=mybir.AluOpType.is_equal,
                        )
                        nc.tensor.matmul(
                            out=agg[:, t4 * P : (t4 + 1) * P],
                            lhsT=ed[:, k * P : (k + 1) * P],
                            rhs=sel[:],
                            start=(k == 0),
                            stop=(k == K - 1),
                        )
                aggS = apool.tile([P, SN], BF16, tag="aggS")
                nc.vector.tensor_copy(aggS[:], agg[:])

                aT = None
                for L in (1, 2, 3):
                    z = psz.tile([P, SN], F32, tag="z")
                    nc.tensor.matmul(
                        out=z[:], lhsT=brow[L][:], rhs=ones_row[:],
                        start=True, stop=False,
                    )
                    if L == 1:
                        nc.tensor.matmul(
                            out=z[:], lhsT=W["w1a"][:], rhs=xT[:],
                            start=False, stop=False,
                        )
                        nc.tensor.matmul(
                            out=z[:], lhsT=W["w1b"][:], rhs=aggS[:],
                            start=False, stop=True,
                        )
                    else:
                        nc.tensor.matmul(
                            out=z[:], lhsT=W[f"w{L}"][:], rhs=aT[:],
                            start=False, stop=True,
                        )
                    # PSUM has one DVE read port: copy z to SBUF bf16 first,
                    # then square in 2x bf16 mode.
                    zc = sqpool.tile([P, SN], BF16, tag="zc")
                    nc.vector.tensor_copy(zc[:], z[:])
                    sq = sqpool.tile([P, SN], BF16, tag="sq")
                    nc.vector.tensor_tensor(
                        sq[:], zc[:], zc[:], op=mybir.AluOpType.mult
                    )
                    var = psvar.tile([P, SN], F32, tag="var")
                    nc.tensor.matmul(
                        out=var[:], lhsT=ones_rep[:], rhs=sq[:],
                        start=True, stop=True,
                    )
                    # rsig = exp(-0.5 * ln(var/H + eps)) broadcast over h
                    lnv = rspool.tile([P, SN], F32, tag="lnv")
                    nc.scalar.activation(
                        lnv[:], var[:], mybir.ActivationFunctionType.Ln,
                        bias=eps[:, 0:1], scale=1.0 / H,
                    )
                    rsig = rspool.tile([P, SN], BF16, tag="rsig")
                    nc.scalar.activation(
                        rsig[:], lnv[:], mybir.ActivationFunctionType.Exp,
                        scale=-0.5,
                    )
                    # zn on GPSIMD (all-SBUF operands) to offload DVE
                    zn = znpool.tile([P, SN], BF16, tag="zn")
                    nc.gpsimd.tensor_tensor(
                        zn[:], zc[:], rsig[:], op=mybir.AluOpType.mult
                    )
                    # ssp(y) = ln(0.5*exp(y) + 0.5), y = g*zn + be; includes
                    # the -log2 shift. |zn| <= sqrt(127) so exp cannot overflow.
                    ez = ezpool.tile([P, SN], F32, tag="ez")
                    nc.scalar.activation(
                        ez[:], zn[:], mybir.ActivationFunctionType.Exp,
                        bias=V[f"be{L}"], scale=V[f"g{L}"],
                    )
                    hT = hpool.tile([P, SN], BF16, tag="hT")
                    nc.scalar.activation(
                        hT[:], ez[:], mybir.ActivationFunctionType.Ln,
                        bias=half[:, 0:1], scale=0.5,
                    )
                    aT = hT
                nc.sync.dma_start(
                    out=out_h[:, st * SN : (st + 1) * SN], in_=aT[:]
                )

    if not nc.is_finalized():
        nc.finalize()
    return nc


def kernel(
    x, edge_index, edge_attr,
    W1, b1, g1, be1, W2, b2, g2, be2, W3, b3, g3, be3,
):
    global LAST_RESULT
    W1 = np.asarray(W1, np.float32)
    W2 = np.asarray(W2, np.float32)
    W3 = np.asarray(W3, np.float32)
    b1 = np.asarray(b1, np.float32)
    b2 = np.asarray(b2, np.float32)
    b3 = np.asarray(b3, np.float32)

    # Fold the LayerNorm mean into weights/biases: W' = W - rowmean, so the
    # matmul output is exactly mean-centered over the hidden dim.
    W1c = W1 - W1.mean(axis=1, keepdims=True)
    W2c = W2 - W2.mean(axis=1, keepdims=True)
    W3c = W3 - W3.mean(axis=1, keepdims=True)
    b1c = b1 - b1.mean()
    b2c = b2 - b2.mean()
    b3c = b3 - b3.mean()

    K, per_core = _host_prep(x, edge_index, edge_attr)
    nc = _build_program(K)

    vecs = np.stack(
        [np.asarray(v, np.float32) for v in (g1, g2, g3, be1, be2, be3)],
        axis=1,
    )  # [128, 6], column order must match VIDX in _build_program
    shared = {
        "w1a": np.ascontiguousarray(W1c[:P]).astype(ml_dtypes.bfloat16),
        "w1b": np.ascontiguousarray(W1c[P:]).astype(ml_dtypes.bfloat16),
        "w2": W2c.astype(ml_dtypes.bfloat16),
        "w3": W3c.astype(ml_dtypes.bfloat16),
        "b1": b1c.reshape(1, P).astype(ml_dtypes.bfloat16),
        "b2": b2c.reshape(1, P).astype(ml_dtypes.bfloat16),
        "b3": b3c.reshape(1, P).astype(ml_dtypes.bfloat16),
        "vecs": np.ascontiguousarray(vecs),
        "iota": np.ascontiguousarray(
            np.broadcast_to(
                np.arange(P, dtype=np.float32).astype(ml_dtypes.bfloat16), (P, P)
            )
        ),
    }
    in_maps = [
        {"edges": pay_c, "cols": col_c, "xt": xt_c, **shared}
        for (pay_c, col_c, xt_c) in per_core
    ]

    trace = bool(int(os.environ.get("KERNEL_TRACE", "0")))
    res = run_bass_kernel_spmd(nc, in_maps, core_ids=list(range(NC)), trace=trace)
    LAST_RESULT = res

    out = np.concatenate(
        [np.asarray(r["out"], dtype=np.float32).T for r in res.results], axis=0
    )
    return np.ascontiguousarray(out[:N])


# revision 12
# speedup vs baseline: 2.7972x; 1.8936x over previous
"""Trainium2 Bass kernel for nn_NodeModel (GNN message passing + 3-layer node MLP).

Strategy (node-parallel, 8 cores), v2 — transpose-free [h, node] dataflow:
  - Host: sort edges by destination node, bucket into 128-node tiles, pad each
    tile's edge list to K chunks of 128 edges. Nodes sharded contiguously
    across 8 cores (12800 padded nodes each, 25 super-tiles of 512).
  - LayerNorm mean is folded into the weights on host (W' = W - rowmean(W),
    b' = b - mean(b)) so every matmul output is already mean-centered.
  - Per 512-node super-tile on device:
      * aggregation: one-hot sel built by tensor_scalar(iota == col) on
        DVE/GPSIMD, chunk matmuls accumulate aggT[h, n] in PSUM.
      * per layer: bias seeded into PSUM by a rank-1 matmul (b'_row x ones),
        z_c = W'.T @ aT accumulated on top; var broadcast to all partitions
        via an all-ones stationary matmul of sq = z_c^2; rsig = exp(-.5*ln(.));
        zn = z_c * rsig; ssp via two ACT ops Exp(g*x+be), Ln(.5x+.5).
    Everything stays [h, node]; no PE transposes, no bn_stats.
  - ssp's -log2 appears exactly as ln(0.5 e^y + 0.5); output written bf16.
"""

import os
import sys

import numpy as np

sys.path.insert(0, "/opt/trn_rl_repo")

import bass_rust as _bass_rust
import ml_dtypes

from concourse import bacc, bass, hw_specs, mybir
from concourse import tile as tile_mod
from concourse.bass_utils import run_bass_kernel_spmd


class _Bacc(bacc.Bacc):
    """Bacc with the ACT table chooser pinned to the single function set
    that holds Ln+Exp+Copy+Identity. The default greedy chooser alternates
    between per-func sets, costing a ~1.3us ACT_TABLE_LOAD per switch."""

    def insert_act_table_loads(self):
        has_activation = any(
            isinstance(i, mybir.InstActivation)
            for b in self.main_func.blocks
            for i in b.instructions
        )
        if not has_activation:
            return
        keep = "natural_log_exp_and_others"
        tables = [
            (n, (s if n == keep else set()))
            for n, s in hw_specs.get_activation_tables(self.m.arch).items()
        ]
        _bass_rust.insert_act_table_loads(self, tables)


LOG2 = float(np.log(2.0))
N, E, H = 100000, 600000, 128
NC = 8
P = 128
SN = 512                 # nodes per super-tile
TPS = SN // P            # 128-node tiles per super-tile (4)
SPC = 25                 # super-tiles per core
TPC = SPC * TPS          # 128-node tiles per core (100)
NPC = TPC * P            # nodes per core (12800)
NPAD = NPC * NC          # padded node count (102400)
NT = NPAD // P           # total node tiles (800)

F32 = mybir.dt.float32
BF16 = mybir.dt.bfloat16

LAST_RESULT = None  # BassKernelResults of the most recent run (for profiling)


def _host_prep(x, edge_index, edge_attr):
    col = np.asarray(edge_index)[1].astype(np.int64)
    ea = np.ascontiguousarray(np.asarray(edge_attr, dtype=np.float32))
    order = np.argsort(col, kind="stable")
    col_s = col[order]
    tile_of = col_s >> 7
    counts = np.bincount(tile_of, minlength=NT)
    K = int(np.ceil(counts.max() / P))
    S = K * P
    starts = np.zeros(NT + 1, np.int64)
    starts[1:] = np.cumsum(counts)
    pos = np.arange(E) - starts[tile_of]
    slot = tile_of * S + pos
    slot_edge = np.zeros(NT * S, np.int64)
    slot_edge[slot] = order
    col_local = np.full(NT * S, 128.0, np.float32)
    col_local[slot] = (col_s & 127).astype(np.float32)
    payload = ea[slot_edge]  # [NT*S, H]

    x_pad = np.zeros((NPAD, H), np.float32)
    x_pad[:N] = np.asarray(x, dtype=np.float32)

    per_core = []
    for c in range(NC):
        r0, r1 = c * TPC * S, (c + 1) * TPC * S
        # ed rows = edge position within chunk, cols = (tile-chunk, h)
        pay_c = np.ascontiguousarray(
            payload[r0:r1]
            .reshape(TPC, K, P, H)
            .transpose(0, 2, 1, 3)
            .reshape(TPC * P, K * H)
            .astype(ml_dtypes.bfloat16)
        )
        col_c = np.ascontiguousarray(
            col_local[r0:r1].reshape(TPC, K, P).transpose(2, 0, 1).reshape(P, TPC * K)
        )
        # xT: [h, node] per core
        xt_c = np.ascontiguousarray(
            x_pad[c * NPC : (c + 1) * NPC].T.astype(ml_dtypes.bfloat16)
        )
        per_core.append((pay_c, col_c, xt_c))
    return K, per_core


def _build_program(K):
    # Bacc (not raw Bass): its compile pass splits multi-semaphore waits into
    # event-semaphore chains — walrus codegen allows only 1 wait per
    # instruction on this toolchain.
    nc = _Bacc("TRN2", target_bir_lowering=False, debug=False, num_devices=NC)

    edges_h = nc.dram_tensor("edges", [TPC * P, K * P], BF16, kind="ExternalInput")
    cols_h = nc.dram_tensor("cols", [P, TPC * K], F32, kind="ExternalInput")
    xt_h = nc.dram_tensor("xt", [P, NPC], BF16, kind="ExternalInput")
    w_h = {
        name: nc.dram_tensor(name, [P, P], BF16, kind="ExternalInput")
        for name in ("w1a", "w1b", "w2", "w3")
    }
    # bias rows b1',b2',b3' (already mean-centered) as [1,128] for rank-1 seed
    brow_h = {
        i: nc.dram_tensor(f"b{i}", [1, P], BF16, kind="ExternalInput")
        for i in (1, 2, 3)
    }
    # g1..g3, be1..be3 packed as columns of one tensor
    vecs_h = nc.dram_tensor("vecs", [P, 6], F32, kind="ExternalInput")
    # iota ramp tiled K times along free dim, for wide one-hot builds
    iota_h = nc.dram_tensor("iota", [P, K * P], F32, kind="ExternalInput")
    out_h = nc.dram_tensor("out", [P, NPC], BF16, kind="ExternalOutput")
    VIDX = {n: i for i, n in enumerate(("g1", "g2", "g3", "be1", "be2", "be3"))}

    n_st = int(os.environ.get("KERNEL_SPC", str(SPC)))

    with tile_mod.TileContext(nc) as tc:
        with (
            tc.tile_pool(name="const", bufs=1) as cpool,
            tc.tile_pool(name="edges", bufs=8) as epool,
            tc.tile_pool(name="xin", bufs=3) as xpool,
            tc.tile_pool(name="sel", bufs=6) as selpool,
            tc.tile_pool(name="aggs", bufs=3) as apool,
            tc.tile_pool(name="sq", bufs=3) as sqpool,
            tc.tile_pool(name="rs", bufs=3) as rspool,
            tc.tile_pool(name="zn", bufs=3) as znpool,
            tc.tile_pool(name="ez", bufs=3) as ezpool,
            tc.tile_pool(name="hout", bufs=4) as hpool,
            tc.tile_pool(name="psagg", bufs=2, space="PSUM") as pagg,
            tc.tile_pool(name="psz", bufs=3, space="PSUM") as psz,
            tc.tile_pool(name="psvar", bufs=3, space="PSUM") as psvar,
        ):
            iota = cpool.tile_from(iota_h[:])
            cols = cpool.tile_from(cols_h[:])
            W = {k: cpool.tile_from(h[:], name=f"w_{k}") for k, h in w_h.items()}
            brow = {i: cpool.tile_from(h[:], name=f"b_{i}") for i, h in brow_h.items()}
            vecs = cpool.tile_from(vecs_h[:])
            V = {n: vecs[:, i : i + 1] for n, i in VIDX.items()}
            ones_rep = cpool.tile([P, P], BF16)
            nc.gpsimd.memset(ones_rep[:], 1.0)
            ones_row = cpool.tile([1, SN], BF16)
            nc.gpsimd.memset(ones_row[:], 1.0)
            eps = cpool.tile([P, 1], F32)
            nc.gpsimd.memset(eps[:], 1e-5)
            half = cpool.tile([P, 1], F32)
            nc.gpsimd.memset(half[:], 0.5)

            for st in range(n_st):
                xT = xpool.tile([P, SN], BF16, tag="xT")
                nc.sync.dma_start(out=xT[:], in_=xt_h[:, st * SN : (st + 1) * SN])

                agg = pagg.tile([P, SN], F32, tag="agg")
                for t4 in range(TPS):
                    t = st * TPS + t4
                    ed = epool.tile([P, K * P], BF16, tag="ed")
                    nc.sync.dma_start(out=ed[:], in_=edges_h[t * P : (t + 1) * P, :])
                    # one wide one-hot build per tile: sel[e, k*128+n] =
                    # (cols[e, t*K+k] == n), via 3D broadcast APs
                    sel = selpool.tile([P, K * P], BF16, tag="sel")
                    # is_equal only lowers on DVE (Pool rejects compare ops)
                    nc.vector.tensor_tensor(
                        sel[:].rearrange("p (k n) -> p k n", k=K),
                        cols[:, t * K : (t + 1) * K].to_broadcast([P, K, P]),
                        iota[:].rearrange("p (k n) -> p k n", k=K),
                        op=mybir.AluOpType.is_equal,
                    )
                    for k in range(K):
                        nc.tensor.matmul(
                            out=agg[:, t4 * P : (t4 + 1) * P],
                            lhsT=ed[:, k * P : (k + 1) * P],
                            rhs=sel[:, k * P : (k + 1) * P],
                            start=(k == 0),
                            stop=(k == K - 1),
                        )
                aggS = apool.tile([P, SN], BF16, tag="aggS")
                nc.vector.tensor_copy(aggS[:], agg[:])

                aT = None
                for L in (1, 2, 3):
                    z = psz.tile([P, SN], F32, tag="z")
                    nc.tensor.matmul(
                        out=z[:], lhsT=brow[L][:], rhs=ones_row[:],
                        start=True, stop=False,
                    )
                    if L == 1:
                        nc.tensor.matmul(
                            out=z[:], lhsT=W["w1a"][:], rhs=xT[:],
                            start=False, stop=False,
                        )
                        nc.tensor.matmul(
                            out=z[:], lhsT=W["w1b"][:], rhs=aggS[:],
                            start=False, stop=True,
                        )
                    else:
                        nc.tensor.matmul(
                            out=z[:], lhsT=W[f"w{L}"][:], rhs=aT[:],
                            start=False, stop=True,
                        )
                    # PSUM has one DVE read port: copy z to SBUF bf16 first,
                    # then square in 2x bf16 mode.
                    zc = sqpool.tile([P, SN], BF16, tag="zc")
                    nc.vector.tensor_copy(zc[:], z[:])
                    # square on GPSIMD (SBUF-only engine) to offload DVE
                    sq = sqpool.tile([P, SN], BF16, tag="sq")
                    nc.gpsimd.tensor_tensor(
                        sq[:], zc[:], zc[:], op=mybir.AluOpType.mult
                    )
                    var = psvar.tile([P, SN], F32, tag="var")
                    nc.tensor.matmul(
                        out=var[:], lhsT=ones_rep[:], rhs=sq[:],
                        start=True, stop=True,
                    )
                    # rsig = exp(-0.5 * ln(var/H + eps)) broadcast over h
                    lnv = rspool.tile([P, SN], F32, tag="lnv")
                    nc.scalar.activation(
                        lnv[:], var[:], mybir.ActivationFunctionType.Ln,
                        bias=eps[:, 0:1], scale=1.0 / H,
                    )
                    rsig = rspool.tile([P, SN], BF16, tag="rsig")
                    nc.scalar.activation(
                        rsig[:], lnv[:], mybir.ActivationFunctionType.Exp,
                        scale=-0.5,
                    )
                    # zn on GPSIMD (all-SBUF operands) to offload DVE
                    zn = znpool.tile([P, SN], BF16, tag="zn")
                    nc.gpsimd.tensor_tensor(
                        zn[:], zc[:], rsig[:], op=mybir.AluOpType.mult
                    )
                    # ssp(y) = ln(0.5*exp(y) + 0.5), y = g*zn + be; includes
                    # the -log2 shift. |zn| <= sqrt(127) so exp cannot overflow.
                    ez = ezpool.tile([P, SN], F32, tag="ez")
                    nc.scalar.activation(
                        ez[:], zn[:], mybir.ActivationFunctionType.Exp,
                        bias=V[f"be{L}"], scale=V[f"g{L}"],
                    )
                    hT = hpool.tile([P, SN], BF16, tag="hT")
                    nc.scalar.activation(
                        hT[:], ez[:], mybir.ActivationFunctionType.Ln,
                        bias=half[:, 0:1], scale=0.5,
                    )
                    aT = hT
                nc.sync.dma_start(
                    out=out_h[:, st * SN : (st + 1) * SN], in_=aT[:]
                )

    if not nc.is_finalized():
        nc.finalize()
    return nc


def kernel(
    x, edge_index, edge_attr,
    W1, b1, g1, be1, W2, b2, g2, be2, W3, b3, g3, be3,
):
    global LAST_RESULT
    W1 = np.asarray(W1, np.float32)
    W2 = np.asarray(W2, np.float32)
    W3 = np.asarray(W3, np.float32)
    b1 = np.asarray(b1, np.float32)
    b2 = np.asarray(b2, np.float32)
    b3 = np.asarray(b3, np.float32)

    # Fold the LayerNorm mean into weights/biases: W' = W - rowmean, so the
    # matmul output is exactly mean-centered over the hidden dim.
    W1c = W1 - W1.mean(axis=1, keepdims=True)
    W2c = W2 - W2.mean(axis=1, keepdims=True)
    W3c = W3 - W3.mean(axis=1, keepdims=True)
    b1c = b1 - b1.mean()
    b2c = b2 - b2.mean()
    b3c = b3 - b3.mean()

    K, per_core = _host_prep(x, edge_index, edge_attr)
    nc = _build_program(K)

    vecs = np.stack(
        [np.asarray(v, np.float32) for v in (g1, g2, g3, be1, be2, be3)],
        axis=1,
    )  # [128, 6], column order must match VIDX in _build_program
    shared = {
        "w1a": np.ascontiguousarray(W1c[:P]).astype(ml_dtypes.bfloat16),
        "w1b": np.ascontiguousarray(W1c[P:]).astype(ml_dtypes.bfloat16),
        "w2": W2c.astype(ml_dtypes.bfloat16),
        "w3": W3c.astype(ml_dtypes.bfloat16),
        "b1": b1c.reshape(1, P).astype(ml_dtypes.bfloat16),
        "b2": b2c.reshape(1, P).astype(ml_dtypes.bfloat16),
        "b3": b3c.reshape(1, P).astype(ml_dtypes.bfloat16),
        "vecs": np.ascontiguousarray(vecs),
        "iota": np.ascontiguousarray(
            np.broadcast_to(
                np.tile(np.arange(P, dtype=np.float32), K), (P, K * P)
            )
        ),
    }
    in_maps = [
        {"edges": pay_c, "cols": col_c, "xt": xt_c, **shared}
        for (pay_c, col_c, xt_c) in per_core
    ]

    trace = bool(int(os.environ.get("KERNEL_TRACE", "0")))
    res = run_bass_kernel_spmd(nc, in_maps, core_ids=list(range(NC)), trace=trace)
    LAST_RESULT = res

    out = np.concatenate(
        [np.asarray(r["out"], dtype=np.float32).T for r in res.results], axis=0
    )
    return np.ascontiguousarray(out[:N])
